# revision 28
# baseline (speedup 1.0000x reference)
"""Multi-head self-attention (B=4, S=2048, E=1024, H=16, causal) on 8 NeuronCores.

Round-3 configuration (best measured: 371us): AV one round behind scores,
normalization at phase end via broadcast-then-approx-reciprocal, one filler
per round, 20 warm-up matmuls, qc order (3,0,1,2), single-queue x DMA.
"""

import numpy as np
import ml_dtypes

B, S, E, H, D = 4, 2048, 1024, 16, 64
HPC = 8          # heads per core
DC = HPC * D     # 512 sharded feature cols per core
EC = E // 128    # 8 e-chunks
TT = S // 128    # 16 token tiles
QCH = S // 512   # 4 query chunks
NB = S // 128    # 16 key blocks

BF16 = ml_dtypes.bfloat16

_CACHE = {}


def _build():
    import concourse.tile as tile
    from concourse import bacc, mybir

    F32 = mybir.dt.float32
    BF = mybir.dt.bfloat16
    AF = mybir.ActivationFunctionType
    ALU = mybir.AluOpType

    nc = bacc.Bacc("TRN2", target_bir_lowering=False, debug=False, num_devices=8)

    xT_d = nc.dram_tensor("xT", [EC, 128, S], BF, kind="ExternalInput")
    wq_d = nc.dram_tensor("wq", [EC, 128, DC], BF, kind="ExternalInput")
    wk_d = nc.dram_tensor("wk", [EC, 128, DC], BF, kind="ExternalInput")
    wv_d = nc.dram_tensor("wv", [EC, 128, DC], BF, kind="ExternalInput")
    wo_d = nc.dram_tensor("wo", [DC // 128, 128, E], BF, kind="ExternalInput")
    bq_d = nc.dram_tensor("bq", [128, 4], F32, kind="ExternalInput")
    bk_d = nc.dram_tensor("bk", [128, 4], F32, kind="ExternalInput")
    bvb_d = nc.dram_tensor("bvb", [128, DC], F32, kind="ExternalInput")
    mask_d = nc.dram_tensor("mask", [128, 128], BF, kind="ExternalInput")
    out_d = nc.dram_tensor("out", [TT, 128, E], F32, kind="ExternalOutput")

    with tile.TileContext(nc) as tc:
        with tc.tile_pool(name="const", bufs=1) as cp, \
             tc.tile_pool(name="expp", bufs=4) as expp, \
             tc.tile_pool(name="work", bufs=2) as wp, \
             tc.tile_pool(name="ps_s", bufs=2, space="PSUM") as ps_s, \
             tc.tile_pool(name="ps_av", bufs=2, space="PSUM") as ps_av, \
             tc.tile_pool(name="ps_w", bufs=2, space="PSUM") as ps_w:

            # ---- PE warm-up during the input-DMA window. Full 128x128
            # stationary: a 64-row warm-up leaves 3/4 of the array idle and
            # the HAM activity monitor never flips to the 2.4 GHz clock ----
            wu = cp.tile([128, 512], BF, tag="wu", name="wu")
            nc.vector.memset(wu[:], 0.125)
            for _ in range(24):
                pw = ps_w.tile([128, 512], F32, tag="psw", name="psw")
                nc.tensor.matmul(pw[:], wu[:, 0:128], wu[:],
                                 start=True, stop=True)

            # ---- persistent SBUF tensors + input DMAs ----
            xT = [cp.tile([128, S], BF, tag=f"xT{k}", name=f"xT{k}") for k in range(EC)]
            wq = [cp.tile([128, DC], BF, tag=f"wq{k}", name=f"wq{k}") for k in range(EC)]
            wk = [cp.tile([128, DC], BF, tag=f"wk{k}", name=f"wk{k}") for k in range(EC)]
            wv = [cp.tile([128, DC], BF, tag=f"wv{k}", name=f"wv{k}") for k in range(EC)]
            wo = [cp.tile([128, E], BF, tag=f"wo{k}", name=f"wo{k}") for k in range(DC // 128)]
            # x split across both DMA queues (one queue alone is the ~20us
            # critical path); wq/wv on one queue, wk on the other, ordered by
            # first use so attention can start ~15us in
            bq = cp.tile([128, 4], F32, tag="bq", name="bq")
            bk = cp.tile([128, 4], F32, tag="bk", name="bk")
            bvb = cp.tile([128, DC], F32, tag="bvb", name="bvb")
            mask = cp.tile([128, 128], BF, tag="mask", name="mask")
            nc.sync.dma_start(bq[:], bq_d.ap())
            nc.sync.dma_start(bk[:], bk_d.ap())
            for k in range(EC):
                q = nc.sync if k % 2 == 0 else nc.gpsimd
                q.dma_start(xT[k][:], xT_d.ap()[k])
            for k in range(EC):
                nc.gpsimd.dma_start(wq[k][:], wq_d.ap()[k])
                nc.sync.dma_start(wk[k][:], wk_d.ap()[k])
            for k in range(EC):
                nc.gpsimd.dma_start(wv[k][:], wv_d.ap()[k])
            nc.sync.dma_start(bvb[:], bvb_d.ap())
            nc.sync.dma_start(mask[:], mask_d.ap())
            for k in range(DC // 128):
                nc.sync.dma_start(wo[k][:], wo_d.ap()[k])
            ones = cp.tile([65, 64], BF, tag="ones", name="ones")
            nc.any.memset(ones[:], 1.0)

            QT = [cp.tile([128, S], BF, tag=f"QT{t}", name=f"QT{t}") for t in range(4)]
            KT = [cp.tile([128, S], BF, tag=f"KT{t}", name=f"KT{t}") for t in range(4)]
            V = [cp.tile([128, HPC, 66], BF, tag=f"V{s}", name=f"V{s}") for s in range(TT)]
            AOT = [cp.tile([128, S], BF, tag=f"AOT{t}", name=f"AOT{t}") for t in range(4)]

            filler = []

            def proj_group(w_sb, b_sb, dst, t, qc):
                def emit():
                    ps = ps_w.tile([128, 512], F32, tag="psw", name="psw")
                    for k in range(EC):
                        nc.tensor.matmul(
                            ps[:],
                            w_sb[k][:, t * 128:(t + 1) * 128],
                            xT[k][:, qc * 512:(qc + 1) * 512],
                            start=(k == 0), stop=(k == EC - 1))
                    nc.vector.tensor_scalar(
                        dst[t][:, qc * 512:(qc + 1) * 512], ps[:],
                        b_sb[:, t:t + 1], None, ALU.add)
                return emit

            def v_group(s):
                def emit():
                    ps = ps_w.tile([128, 512], F32, tag="psw", name="psw")
                    for k in range(EC):
                        nc.tensor.matmul(
                            ps[:],
                            xT[k][:, s * 128:(s + 1) * 128],
                            wv[k][:],
                            start=(k == 0), stop=(k == EC - 1))
                    nc.vector.tensor_tensor(
                        V[s][:, :, 0:64],
                        ps[:].rearrange("p (h d) -> p h d", d=64),
                        bvb[:].rearrange("p (h d) -> p h d", d=64),
                        ALU.add)
                    nc.any.memset(V[s][:, :, 64:65], 1.0)
                return emit

            def d_group(s):
                def emit():
                    osb = wp.tile([128, E], F32, tag="osb", name="osb")
                    for n in range(2):
                        ps = ps_w.tile([128, 512], F32, tag="psw", name="psw")
                        for k in range(DC // 128):
                            nc.tensor.matmul(
                                ps[:],
                                AOT[k][:, s * 128:(s + 1) * 128],
                                wo[k][:, n * 512:(n + 1) * 512],
                                start=(k == 0), stop=(k == DC // 128 - 1))
                        nc.vector.tensor_copy(out=osb[:, n * 512:(n + 1) * 512],
                                              in_=ps[:])
                    nc.sync.dma_start(out_d.ap()[s], osb[:])
                return emit

            # Minimal up-front work: the first attention phase (qc=3, hp=0)
            # needs only QT[0] of the qc=3 query chunk and KT[0] of the first
            # key blocks. Everything else is a filler, demand-forced just
            # before its consumer, so attention starts as soon as the first
            # weight chunks land instead of after all eight t0 groups.
            proj_group(wq, bq, QT, 0, 3)()
            proj_group(wk, bk, KT, 0, 0)()
            for qc in (1, 2, 3):
                filler.append(("kt0", qc, proj_group(wk, bk, KT, 0, qc)))
            for s in range(TT):
                filler.append(("v", s, v_group(s)))
            for qc in (0, 1, 2):
                filler.append(("qt0", qc, proj_group(wq, bq, QT, 0, qc)))
            for t in range(1, 4):
                for qc in range(QCH):
                    filler.append(("qkt", t, proj_group(wq, bq, QT, t, qc)))
                    filler.append(("qkt", t, proj_group(wk, bk, KT, t, qc)))

            def emit_filler_until(pred_drop):
                keep = []
                for item in filler:
                    if pred_drop(item):
                        item[2]()
                    else:
                        keep.append(item)
                filler[:] = keep

            def emit_some_filler(n):
                for _ in range(min(n, len(filler))):
                    filler.pop(0)[2]()

            for qc in (3, 0, 1, 2):
                nkb = 4 * qc + 4
                for hp in range(4):
                    emit_filler_until(lambda it: it[0] == "qt0"
                                      and it[1] == qc)
                    emit_filler_until(lambda it: it[0] == "qkt" and it[1] <= hp)
                    hA, hB = 2 * hp, 2 * hp + 1
                    pav = {}
                    pav[hA] = ps_av.tile([128, 512], F32, tag="pav", name="pav")
                    pav[hB] = ps_av.tile([128, 512], F32, tag="pav", name="pav")

                    def emit_av(ex, kbs_offs):
                        for h in (hA, hB):
                            for i, kb, off in kbs_offs:
                                nc.tensor.matmul(
                                    pav[h][0:65, off:512],
                                    V[kb][:, h, 0:65],
                                    ex[h][:, i, off:512],
                                    start=(kb == 0), stop=(kb == nkb - 1))

                    pend = None
                    for s0 in range(0, nkb, 2):
                        kbs = list(range(s0, min(s0 + 2, nkb)))
                        emit_filler_until(
                            lambda it: it[0] == "kt0"
                            and it[1] <= kbs[-1] // 4)
                        pss = {h: ps_s.tile([128, 2, 512], F32, tag="pss",
                                            name="pss")
                               for h in (hA, hB)}
                        ex = {h: expp.tile([128, 2, 512], BF,
                                           tag=f"ex{h % 2}", name="ex")
                              for h in (hA, hB)}
                        offs = {}
                        for i, kb in enumerate(kbs):
                            dj = kb - 4 * qc
                            off = 128 * dj if dj > 0 else 0
                            offs[kb] = off
                            for h, r in ((hA, 0), (hB, 64)):
                                nc.tensor.matmul(
                                    pss[h][:, i, off:512],
                                    KT[hp][r:r + 64, kb * 128:(kb + 1) * 128],
                                    QT[hp][r:r + 64,
                                           qc * 512 + off:(qc + 1) * 512],
                                    start=True, stop=True)
                        for h in (hA, hB):
                            if kbs[-1] < 4 * qc:
                                nc.scalar.activation(
                                    ex[h][:, 0:len(kbs), :],
                                    pss[h][:, 0:len(kbs), :],
                                    AF.Exp, scale=0.125)
                            else:
                                for i, kb in enumerate(kbs):
                                    dj = kb - 4 * qc
                                    off = offs[kb]
                                    nc.scalar.activation(
                                        ex[h][:, i, off:512],
                                        pss[h][:, i, off:512],
                                        AF.Exp, scale=0.125)
                                    if dj >= 0:
                                        nc.vector.tensor_tensor(
                                            ex[h][:, i, off:off + 128],
                                            ex[h][:, i, off:off + 128],
                                            mask[:], ALU.mult)
                        emit_some_filler(1)
                        emit_filler_until(
                            lambda it: it[0] == "v" and it[1] <= kbs[-1])
                        if pend is not None:
                            emit_av(*pend)
                        pend = (ex, [(i, kb, offs[kb])
                                     for i, kb in enumerate(kbs)])
                    emit_av(*pend)
                    for h, r in ((hA, 0), (hB, 64)):
                        den = wp.tile([1, 512], BF, tag="den", name="den")
                        nc.vector.tensor_copy(out=den[:],
                                              in_=pav[h][64:65, :])
                        psb = ps_w.tile([128, 512], F32, tag="psw", name="psw")
                        nc.tensor.matmul(psb[0:64, :], ones[0:1, :],
                                         den[:], start=True, stop=True)
                        rcpb = wp.tile([64, 512], F32, tag="rcpb", name="rcpb")
                        nc.vector.reciprocal_approx_fast(out=rcpb[:],
                                                         in_=psb[0:64, :])
                        dst = AOT[hp][r:r + 64, qc * 512:(qc + 1) * 512]
                        nc.vector.tensor_tensor(dst, pav[h][0:64, :],
                                                rcpb[:], ALU.mult)
                for s in range(qc * 4, qc * 4 + 4):
                    filler.append(("d", s, d_group(s)))
            emit_filler_until(lambda it: True)

    nc.compile()
    return nc


def _get_nc():
    if "nc" not in _CACHE:
        _CACHE["nc"] = _build()
    return _CACHE["nc"]


def _shard_inputs(x, Wq, bq, Wk, bk, Wv, bv, Wo):
    """Build the 8 per-core input maps (host-side shard/cast/transpose)."""
    x = np.asarray(x, np.float32)
    mask = np.triu(np.ones((128, 128), np.float32)).astype(BF16)  # [k, q] q>=k
    in_maps = []
    for c in range(8):
        b, hg = divmod(c, 2)
        dc = slice(hg * DC, (hg + 1) * DC)
        xT = np.ascontiguousarray(x[b].T).astype(BF16).reshape(EC, 128, S)
        wq_c = np.ascontiguousarray(Wq[:, dc]).astype(BF16).reshape(EC, 128, DC)
        wk_c = np.ascontiguousarray(Wk[:, dc]).astype(BF16).reshape(EC, 128, DC)
        wv_c = np.ascontiguousarray(Wv[:, dc]).astype(BF16).reshape(EC, 128, DC)
        wo_c = np.ascontiguousarray(Wo[dc, :]).astype(BF16).reshape(DC // 128, 128, E)
        bq_c = np.ascontiguousarray(np.asarray(bq[dc], np.float32).reshape(4, 128).T)
        bk_c = np.ascontiguousarray(np.asarray(bk[dc], np.float32).reshape(4, 128).T)
        bvb_c = np.ascontiguousarray(
            np.tile(np.asarray(bv[dc], np.float32).reshape(1, DC), (128, 1)))
        in_maps.append({
            "xT": xT, "wq": wq_c, "wk": wk_c, "wv": wv_c, "wo": wo_c,
            "bq": bq_c, "bk": bk_c, "bvb": bvb_c, "mask": mask,
        })
    return in_maps


def kernel(x, Wq, bq, Wk, bk, Wv, bv, Wo, bo):
    from concourse.bass_utils import run_bass_kernel_spmd

    nc = _get_nc()
    in_maps = _shard_inputs(x, Wq, bq, Wk, bk, Wv, bv, Wo)
    res = run_bass_kernel_spmd(nc, in_maps, core_ids=list(range(8)))
    bo = np.asarray(bo, np.float32)
    out = np.empty((B, S, E), np.float32)
    for b in range(B):
        p0 = res.results[2 * b]["out"].reshape(S, E)
        p1 = res.results[2 * b + 1]["out"].reshape(S, E)
        out[b] = p0 + p1 + bo
    return out


# revision 30
# speedup vs baseline: 1.1552x; 1.1552x over previous
"""Multi-head self-attention (B=4, S=2048, E=1024, H=16, causal) on 8 NeuronCores.

Round-3 configuration (best measured: 371us): AV one round behind scores,
normalization at phase end via broadcast-then-approx-reciprocal, one filler
per round, 20 warm-up matmuls, qc order (3,0,1,2), single-queue x DMA.
"""

import numpy as np
import ml_dtypes

B, S, E, H, D = 4, 2048, 1024, 16, 64
HPC = 8          # heads per core
DC = HPC * D     # 512 sharded feature cols per core
EC = E // 128    # 8 e-chunks
TT = S // 128    # 16 token tiles
QCH = S // 512   # 4 query chunks
NB = S // 128    # 16 key blocks

BF16 = ml_dtypes.bfloat16

_CACHE = {}


def _build():
    import concourse.tile as tile
    from concourse import bacc, mybir

    F32 = mybir.dt.float32
    BF = mybir.dt.bfloat16
    AF = mybir.ActivationFunctionType
    ALU = mybir.AluOpType

    nc = bacc.Bacc("TRN2", target_bir_lowering=False, debug=False, num_devices=8)

    xT_d = nc.dram_tensor("xT", [EC, 128, S], BF, kind="ExternalInput")
    wq_d = nc.dram_tensor("wq", [EC, 128, DC], BF, kind="ExternalInput")
    wk_d = nc.dram_tensor("wk", [EC, 128, DC], BF, kind="ExternalInput")
    wv_d = nc.dram_tensor("wv", [EC, 128, DC], BF, kind="ExternalInput")
    wo_d = nc.dram_tensor("wo", [DC // 128, 128, E], BF, kind="ExternalInput")
    bq_d = nc.dram_tensor("bq", [128, 4], F32, kind="ExternalInput")
    bk_d = nc.dram_tensor("bk", [128, 4], F32, kind="ExternalInput")
    bvb_d = nc.dram_tensor("bvb", [128, DC], F32, kind="ExternalInput")
    mask_d = nc.dram_tensor("mask", [128, 128], BF, kind="ExternalInput")
    out_d = nc.dram_tensor("out", [TT, 128, E], F32, kind="ExternalOutput")

    with tile.TileContext(nc) as tc:
        with tc.tile_pool(name="const", bufs=1) as cp, \
             tc.tile_pool(name="expp", bufs=4) as expp, \
             tc.tile_pool(name="work", bufs=2) as wp, \
             tc.tile_pool(name="ps_s", bufs=2, space="PSUM") as ps_s, \
             tc.tile_pool(name="ps_av", bufs=2, space="PSUM") as ps_av, \
             tc.tile_pool(name="ps_w", bufs=2, space="PSUM") as ps_w:

            # ---- PE warm-up during the input-DMA window. Full 128x128
            # stationary: a 64-row warm-up engages 1/4 of the array and the
            # HAM activity monitor never flips to the 2.4 GHz clock (hardware
            # measurement: full-array warm-ups flip K=8/8 at ~12us instead of
            # ~35us) ----
            wu = cp.tile([128, 512], BF, tag="wu", name="wu")
            nc.vector.memset(wu[:], 0.125)
            for _ in range(24):
                pw = ps_w.tile([128, 512], F32, tag="psw", name="psw")
                nc.tensor.matmul(pw[:], wu[:, 0:128], wu[:],
                                 start=True, stop=True)

            # ---- persistent SBUF tensors + input DMAs ----
            xT = [cp.tile([128, S], BF, tag=f"xT{k}", name=f"xT{k}") for k in range(EC)]
            wq = [cp.tile([128, DC], BF, tag=f"wq{k}", name=f"wq{k}") for k in range(EC)]
            wk = [cp.tile([128, DC], BF, tag=f"wk{k}", name=f"wk{k}") for k in range(EC)]
            wv = [cp.tile([128, DC], BF, tag=f"wv{k}", name=f"wv{k}") for k in range(EC)]
            wo = [cp.tile([128, E], BF, tag=f"wo{k}", name=f"wo{k}") for k in range(DC // 128)]
            for k in range(EC):
                nc.sync.dma_start(xT[k][:], xT_d.ap()[k])
                nc.gpsimd.dma_start(wq[k][:], wq_d.ap()[k])
                nc.gpsimd.dma_start(wk[k][:], wk_d.ap()[k])
                nc.gpsimd.dma_start(wv[k][:], wv_d.ap()[k])
            for k in range(DC // 128):
                nc.sync.dma_start(wo[k][:], wo_d.ap()[k])
            bq = cp.tile([128, 4], F32, tag="bq", name="bq")
            bk = cp.tile([128, 4], F32, tag="bk", name="bk")
            bvb = cp.tile([128, DC], F32, tag="bvb", name="bvb")
            mask = cp.tile([128, 128], BF, tag="mask", name="mask")
            nc.sync.dma_start(bq[:], bq_d.ap())
            nc.sync.dma_start(bk[:], bk_d.ap())
            nc.sync.dma_start(bvb[:], bvb_d.ap())
            nc.sync.dma_start(mask[:], mask_d.ap())
            ones = cp.tile([65, 64], BF, tag="ones", name="ones")
            nc.any.memset(ones[:], 1.0)

            QT = [cp.tile([128, S], BF, tag=f"QT{t}", name=f"QT{t}") for t in range(4)]
            KT = [cp.tile([128, S], BF, tag=f"KT{t}", name=f"KT{t}") for t in range(4)]
            V = [cp.tile([128, HPC, 66], BF, tag=f"V{s}", name=f"V{s}") for s in range(TT)]
            AOT = [cp.tile([128, S], BF, tag=f"AOT{t}", name=f"AOT{t}") for t in range(4)]

            filler = []

            def proj_group(w_sb, b_sb, dst, t, qc):
                def emit():
                    ps = ps_w.tile([128, 512], F32, tag="psw", name="psw")
                    for k in range(EC):
                        nc.tensor.matmul(
                            ps[:],
                            w_sb[k][:, t * 128:(t + 1) * 128],
                            xT[k][:, qc * 512:(qc + 1) * 512],
                            start=(k == 0), stop=(k == EC - 1))
                    nc.vector.tensor_scalar(
                        dst[t][:, qc * 512:(qc + 1) * 512], ps[:],
                        b_sb[:, t:t + 1], None, ALU.add)
                return emit

            def v_group(s):
                def emit():
                    ps = ps_w.tile([128, 512], F32, tag="psw", name="psw")
                    for k in range(EC):
                        nc.tensor.matmul(
                            ps[:],
                            xT[k][:, s * 128:(s + 1) * 128],
                            wv[k][:],
                            start=(k == 0), stop=(k == EC - 1))
                    nc.vector.tensor_tensor(
                        V[s][:, :, 0:64],
                        ps[:].rearrange("p (h d) -> p h d", d=64),
                        bvb[:].rearrange("p (h d) -> p h d", d=64),
                        ALU.add)
                    nc.any.memset(V[s][:, :, 64:65], 1.0)
                return emit

            def d_group(s):
                def emit():
                    osb = wp.tile([128, E], F32, tag="osb", name="osb")
                    for n in range(2):
                        ps = ps_w.tile([128, 512], F32, tag="psw", name="psw")
                        for k in range(DC // 128):
                            nc.tensor.matmul(
                                ps[:],
                                AOT[k][:, s * 128:(s + 1) * 128],
                                wo[k][:, n * 512:(n + 1) * 512],
                                start=(k == 0), stop=(k == DC // 128 - 1))
                        nc.vector.tensor_copy(out=osb[:, n * 512:(n + 1) * 512],
                                              in_=ps[:])
                    nc.sync.dma_start(out_d.ap()[s], osb[:])
                return emit

            for t in range(4):
                for qc in range(QCH):
                    if t == 0:
                        proj_group(wq, bq, QT, t, qc)()
                        proj_group(wk, bk, KT, t, qc)()
                    else:
                        filler.append(("qkt", t, proj_group(wq, bq, QT, t, qc)))
                        filler.append(("qkt", t, proj_group(wk, bk, KT, t, qc)))
            for s in range(TT):
                if s < 4:
                    v_group(s)()
                else:
                    filler.append(("v", s, v_group(s)))

            def emit_filler_until(pred_drop):
                keep = []
                for item in filler:
                    if pred_drop(item):
                        item[2]()
                    else:
                        keep.append(item)
                filler[:] = keep

            def emit_some_filler(n):
                for _ in range(min(n, len(filler))):
                    filler.pop(0)[2]()

            for qc in (3, 0, 1, 2):
                nkb = 4 * qc + 4
                for hp in range(4):
                    emit_filler_until(lambda it: it[0] == "qkt" and it[1] <= hp)
                    hA, hB = 2 * hp, 2 * hp + 1
                    pav = {}
                    pav[hA] = ps_av.tile([128, 512], F32, tag="pav", name="pav")
                    pav[hB] = ps_av.tile([128, 512], F32, tag="pav", name="pav")

                    def emit_av(ex, kbs_offs):
                        for h in (hA, hB):
                            for i, kb, off in kbs_offs:
                                nc.tensor.matmul(
                                    pav[h][0:65, off:512],
                                    V[kb][:, h, 0:65],
                                    ex[h][:, i, off:512],
                                    start=(kb == 0), stop=(kb == nkb - 1))

                    pend = None
                    for s0 in range(0, nkb, 2):
                        kbs = list(range(s0, min(s0 + 2, nkb)))
                        emit_filler_until(
                            lambda it: it[0] == "v" and it[1] <= kbs[-1])
                        pss = {h: ps_s.tile([128, 2, 512], F32, tag="pss",
                                            name="pss")
                               for h in (hA, hB)}
                        ex = {h: expp.tile([128, 2, 512], BF,
                                           tag=f"ex{h % 2}", name="ex")
                              for h in (hA, hB)}
                        offs = {}
                        for i, kb in enumerate(kbs):
                            dj = kb - 4 * qc
                            off = 128 * dj if dj > 0 else 0
                            offs[kb] = off
                            for h, r in ((hA, 0), (hB, 64)):
                                nc.tensor.matmul(
                                    pss[h][:, i, off:512],
                                    KT[hp][r:r + 64, kb * 128:(kb + 1) * 128],
                                    QT[hp][r:r + 64,
                                           qc * 512 + off:(qc + 1) * 512],
                                    start=True, stop=True)
                        for h in (hA, hB):
                            if kbs[-1] < 4 * qc:
                                nc.scalar.activation(
                                    ex[h][:, 0:len(kbs), :],
                                    pss[h][:, 0:len(kbs), :],
                                    AF.Exp, scale=0.125)
                            else:
                                for i, kb in enumerate(kbs):
                                    dj = kb - 4 * qc
                                    off = offs[kb]
                                    nc.scalar.activation(
                                        ex[h][:, i, off:512],
                                        pss[h][:, i, off:512],
                                        AF.Exp, scale=0.125)
                                    if dj >= 0:
                                        nc.vector.tensor_tensor(
                                            ex[h][:, i, off:off + 128],
                                            ex[h][:, i, off:off + 128],
                                            mask[:], ALU.mult)
                        emit_some_filler(1)
                        if pend is not None:
                            emit_av(*pend)
                        pend = (ex, [(i, kb, offs[kb])
                                     for i, kb in enumerate(kbs)])
                    emit_av(*pend)
                    for h, r in ((hA, 0), (hB, 64)):
                        den = wp.tile([1, 512], BF, tag="den", name="den")
                        nc.vector.tensor_copy(out=den[:],
                                              in_=pav[h][64:65, :])
                        psb = ps_w.tile([128, 512], F32, tag="psw", name="psw")
                        nc.tensor.matmul(psb[0:64, :], ones[0:1, :],
                                         den[:], start=True, stop=True)
                        rcpb = wp.tile([64, 512], F32, tag="rcpb", name="rcpb")
                        nc.vector.reciprocal_approx_fast(out=rcpb[:],
                                                         in_=psb[0:64, :])
                        dst = AOT[hp][r:r + 64, qc * 512:(qc + 1) * 512]
                        nc.vector.tensor_tensor(dst, pav[h][0:64, :],
                                                rcpb[:], ALU.mult)
                for s in range(qc * 4, qc * 4 + 4):
                    filler.append(("d", s, d_group(s)))
            emit_filler_until(lambda it: True)

    nc.compile()
    return nc


def _get_nc():
    if "nc" not in _CACHE:
        _CACHE["nc"] = _build()
    return _CACHE["nc"]


def _shard_inputs(x, Wq, bq, Wk, bk, Wv, bv, Wo):
    """Build the 8 per-core input maps (host-side shard/cast/transpose)."""
    x = np.asarray(x, np.float32)
    mask = np.triu(np.ones((128, 128), np.float32)).astype(BF16)  # [k, q] q>=k
    in_maps = []
    for c in range(8):
        b, hg = divmod(c, 2)
        dc = slice(hg * DC, (hg + 1) * DC)
        xT = np.ascontiguousarray(x[b].T).astype(BF16).reshape(EC, 128, S)
        wq_c = np.ascontiguousarray(Wq[:, dc]).astype(BF16).reshape(EC, 128, DC)
        wk_c = np.ascontiguousarray(Wk[:, dc]).astype(BF16).reshape(EC, 128, DC)
        wv_c = np.ascontiguousarray(Wv[:, dc]).astype(BF16).reshape(EC, 128, DC)
        wo_c = np.ascontiguousarray(Wo[dc, :]).astype(BF16).reshape(DC // 128, 128, E)
        bq_c = np.ascontiguousarray(np.asarray(bq[dc], np.float32).reshape(4, 128).T)
        bk_c = np.ascontiguousarray(np.asarray(bk[dc], np.float32).reshape(4, 128).T)
        bvb_c = np.ascontiguousarray(
            np.tile(np.asarray(bv[dc], np.float32).reshape(1, DC), (128, 1)))
        in_maps.append({
            "xT": xT, "wq": wq_c, "wk": wk_c, "wv": wv_c, "wo": wo_c,
            "bq": bq_c, "bk": bk_c, "bvb": bvb_c, "mask": mask,
        })
    return in_maps


def kernel(x, Wq, bq, Wk, bk, Wv, bv, Wo, bo):
    from concourse.bass_utils import run_bass_kernel_spmd

    nc = _get_nc()
    in_maps = _shard_inputs(x, Wq, bq, Wk, bk, Wv, bv, Wo)
    res = run_bass_kernel_spmd(nc, in_maps, core_ids=list(range(8)))
    bo = np.asarray(bo, np.float32)
    out = np.empty((B, S, E), np.float32)
    for b in range(B):
        p0 = res.results[2 * b]["out"].reshape(S, E)
        p1 = res.results[2 * b + 1]["out"].reshape(S, E)
        out[b] = p0 + p1 + bo
    return out


# revision 31
# speedup vs baseline: 1.1993x; 1.0382x over previous
"""Multi-head self-attention (B=4, S=2048, E=1024, H=16, causal) on 8 NeuronCores.

Round-3 configuration (best measured: 371us): AV one round behind scores,
normalization at phase end via broadcast-then-approx-reciprocal, one filler
per round, 20 warm-up matmuls, qc order (3,0,1,2), single-queue x DMA.
"""

import numpy as np
import ml_dtypes

B, S, E, H, D = 4, 2048, 1024, 16, 64
HPC = 8          # heads per core
DC = HPC * D     # 512 sharded feature cols per core
EC = E // 128    # 8 e-chunks
TT = S // 128    # 16 token tiles
QCH = S // 512   # 4 query chunks
NB = S // 128    # 16 key blocks

BF16 = ml_dtypes.bfloat16

_CACHE = {}


def _build():
    import concourse.tile as tile
    from concourse import bacc, mybir

    F32 = mybir.dt.float32
    BF = mybir.dt.bfloat16
    AF = mybir.ActivationFunctionType
    ALU = mybir.AluOpType

    nc = bacc.Bacc("TRN2", target_bir_lowering=False, debug=False, num_devices=8)

    xT_d = nc.dram_tensor("xT", [EC, 128, S], BF, kind="ExternalInput")
    wq_d = nc.dram_tensor("wq", [EC, 128, DC], BF, kind="ExternalInput")
    wk_d = nc.dram_tensor("wk", [EC, 128, DC], BF, kind="ExternalInput")
    wv_d = nc.dram_tensor("wv", [EC, 128, DC], BF, kind="ExternalInput")
    wo_d = nc.dram_tensor("wo", [DC // 128, 128, E], BF, kind="ExternalInput")
    bq_d = nc.dram_tensor("bq", [128, 4], F32, kind="ExternalInput")
    bk_d = nc.dram_tensor("bk", [128, 4], F32, kind="ExternalInput")
    bvb_d = nc.dram_tensor("bvb", [128, DC], F32, kind="ExternalInput")
    mask_d = nc.dram_tensor("mask", [128, 128], BF, kind="ExternalInput")
    out_d = nc.dram_tensor("out", [TT, 128, E], F32, kind="ExternalOutput")

    with tile.TileContext(nc) as tc:
        with tc.tile_pool(name="const", bufs=1) as cp, \
             tc.tile_pool(name="expp", bufs=4) as expp, \
             tc.tile_pool(name="work", bufs=2) as wp, \
             tc.tile_pool(name="ps_s", bufs=2, space="PSUM") as ps_s, \
             tc.tile_pool(name="ps_av", bufs=2, space="PSUM") as ps_av, \
             tc.tile_pool(name="ps_w", bufs=2, space="PSUM") as ps_w:

            # ---- PE warm-up during the input-DMA window ----
            wu = cp.tile([64, 512], BF, tag="wu", name="wu")
            nc.vector.memset(wu[:], 0.125)
            for _ in range(20):
                pw = ps_w.tile([128, 512], F32, tag="psw", name="psw")
                nc.tensor.matmul(pw[0:64, :], wu[:, 0:64], wu[:],
                                 start=True, stop=True)

            # ---- persistent SBUF tensors + input DMAs ----
            xT = [cp.tile([128, S], BF, tag=f"xT{k}", name=f"xT{k}") for k in range(EC)]
            wq = [cp.tile([128, DC], BF, tag=f"wq{k}", name=f"wq{k}") for k in range(EC)]
            wk = [cp.tile([128, DC], BF, tag=f"wk{k}", name=f"wk{k}") for k in range(EC)]
            wv = [cp.tile([128, DC], BF, tag=f"wv{k}", name=f"wv{k}") for k in range(EC)]
            wo = [cp.tile([128, E], BF, tag=f"wo{k}", name=f"wo{k}") for k in range(DC // 128)]
            for k in range(EC):
                nc.sync.dma_start(xT[k][:], xT_d.ap()[k])
                nc.gpsimd.dma_start(wq[k][:], wq_d.ap()[k])
                nc.gpsimd.dma_start(wk[k][:], wk_d.ap()[k])
                nc.gpsimd.dma_start(wv[k][:], wv_d.ap()[k])
            for k in range(DC // 128):
                nc.sync.dma_start(wo[k][:], wo_d.ap()[k])
            bq = cp.tile([128, 4], F32, tag="bq", name="bq")
            bk = cp.tile([128, 4], F32, tag="bk", name="bk")
            bvb = cp.tile([128, DC], F32, tag="bvb", name="bvb")
            mask = cp.tile([128, 128], BF, tag="mask", name="mask")
            nc.sync.dma_start(bq[:], bq_d.ap())
            nc.sync.dma_start(bk[:], bk_d.ap())
            nc.sync.dma_start(bvb[:], bvb_d.ap())
            nc.sync.dma_start(mask[:], mask_d.ap())
            ones = cp.tile([65, 64], BF, tag="ones", name="ones")
            nc.any.memset(ones[:], 1.0)

            QT = [cp.tile([128, S], BF, tag=f"QT{t}", name=f"QT{t}") for t in range(4)]
            KT = [cp.tile([128, S], BF, tag=f"KT{t}", name=f"KT{t}") for t in range(4)]
            V = [cp.tile([128, HPC, 66], BF, tag=f"V{s}", name=f"V{s}") for s in range(TT)]
            AOT = [cp.tile([128, S], BF, tag=f"AOT{t}", name=f"AOT{t}") for t in range(4)]

            filler = []

            def proj_group(w_sb, b_sb, dst, t, qc):
                def emit():
                    ps = ps_w.tile([128, 512], F32, tag="psw", name="psw")
                    for k in range(EC):
                        nc.tensor.matmul(
                            ps[:],
                            w_sb[k][:, t * 128:(t + 1) * 128],
                            xT[k][:, qc * 512:(qc + 1) * 512],
                            start=(k == 0), stop=(k == EC - 1))
                    nc.vector.tensor_scalar(
                        dst[t][:, qc * 512:(qc + 1) * 512], ps[:],
                        b_sb[:, t:t + 1], None, ALU.add)
                return emit

            def v_group(s):
                def emit():
                    ps = ps_w.tile([128, 512], F32, tag="psw", name="psw")
                    for k in range(EC):
                        nc.tensor.matmul(
                            ps[:],
                            xT[k][:, s * 128:(s + 1) * 128],
                            wv[k][:],
                            start=(k == 0), stop=(k == EC - 1))
                    nc.vector.tensor_tensor(
                        V[s][:, :, 0:64],
                        ps[:].rearrange("p (h d) -> p h d", d=64),
                        bvb[:].rearrange("p (h d) -> p h d", d=64),
                        ALU.add)
                    nc.any.memset(V[s][:, :, 64:65], 1.0)
                return emit

            def d_group(s):
                def emit():
                    osb = wp.tile([128, E], F32, tag="osb", name="osb")
                    for n in range(2):
                        ps = ps_w.tile([128, 512], F32, tag="psw", name="psw")
                        for k in range(DC // 128):
                            nc.tensor.matmul(
                                ps[:],
                                AOT[k][:, s * 128:(s + 1) * 128],
                                wo[k][:, n * 512:(n + 1) * 512],
                                start=(k == 0), stop=(k == DC // 128 - 1))
                        nc.vector.tensor_copy(out=osb[:, n * 512:(n + 1) * 512],
                                              in_=ps[:])
                    nc.sync.dma_start(out_d.ap()[s], osb[:])
                return emit

            for t in range(4):
                for qc in range(QCH):
                    if t == 0:
                        proj_group(wq, bq, QT, t, qc)()
                        proj_group(wk, bk, KT, t, qc)()
                    else:
                        filler.append(("qkt", t, proj_group(wq, bq, QT, t, qc)))
                        filler.append(("qkt", t, proj_group(wk, bk, KT, t, qc)))
            for s in range(TT):
                if s < 4:
                    v_group(s)()
                else:
                    filler.append(("v", s, v_group(s)))

            def emit_filler_until(pred_drop):
                keep = []
                for item in filler:
                    if pred_drop(item):
                        item[2]()
                    else:
                        keep.append(item)
                filler[:] = keep

            def emit_some_filler(n):
                for _ in range(min(n, len(filler))):
                    filler.pop(0)[2]()

            for qc in (3, 0, 1, 2):
                nkb = 4 * qc + 4
                for hp in range(4):
                    emit_filler_until(lambda it: it[0] == "qkt" and it[1] <= hp)
                    hA, hB = 2 * hp, 2 * hp + 1
                    pav = {}
                    pav[hA] = ps_av.tile([128, 512], F32, tag="pav", name="pav")
                    pav[hB] = ps_av.tile([128, 512], F32, tag="pav", name="pav")

                    def emit_av(ex, kbs_offs):
                        for h in (hA, hB):
                            for i, kb, off in kbs_offs:
                                nc.tensor.matmul(
                                    pav[h][0:65, off:512],
                                    V[kb][:, h, 0:65],
                                    ex[h][:, i, off:512],
                                    start=(kb == 0), stop=(kb == nkb - 1))

                    pend = None
                    for s0 in range(0, nkb, 2):
                        kbs = list(range(s0, min(s0 + 2, nkb)))
                        emit_filler_until(
                            lambda it: it[0] == "v" and it[1] <= kbs[-1])
                        pss = {h: ps_s.tile([128, 2, 512], F32, tag="pss",
                                            name="pss")
                               for h in (hA, hB)}
                        ex = {h: expp.tile([128, 2, 512], BF,
                                           tag=f"ex{h % 2}", name="ex")
                              for h in (hA, hB)}
                        offs = {}
                        for i, kb in enumerate(kbs):
                            dj = kb - 4 * qc
                            off = 128 * dj if dj > 0 else 0
                            offs[kb] = off
                            for h, r in ((hA, 0), (hB, 64)):
                                nc.tensor.matmul(
                                    pss[h][:, i, off:512],
                                    KT[hp][r:r + 64, kb * 128:(kb + 1) * 128],
                                    QT[hp][r:r + 64,
                                           qc * 512 + off:(qc + 1) * 512],
                                    start=True, stop=True)
                        for h in (hA, hB):
                            if kbs[-1] < 4 * qc:
                                nc.scalar.activation(
                                    ex[h][:, 0:len(kbs), :],
                                    pss[h][:, 0:len(kbs), :],
                                    AF.Exp, scale=0.125)
                            else:
                                for i, kb in enumerate(kbs):
                                    dj = kb - 4 * qc
                                    off = offs[kb]
                                    nc.scalar.activation(
                                        ex[h][:, i, off:512],
                                        pss[h][:, i, off:512],
                                        AF.Exp, scale=0.125)
                                    if dj >= 0:
                                        nc.vector.tensor_tensor(
                                            ex[h][:, i, off:off + 128],
                                            ex[h][:, i, off:off + 128],
                                            mask[:], ALU.mult)
                        emit_some_filler(1)
                        if pend is not None:
                            emit_av(*pend)
                        pend = (ex, [(i, kb, offs[kb])
                                     for i, kb in enumerate(kbs)])
                    emit_av(*pend)
                    for h, r in ((hA, 0), (hB, 64)):
                        den = wp.tile([1, 512], BF, tag="den", name="den")
                        nc.vector.tensor_copy(out=den[:],
                                              in_=pav[h][64:65, :])
                        psb = ps_w.tile([128, 512], F32, tag="psw", name="psw")
                        nc.tensor.matmul(psb[0:64, :], ones[0:1, :],
                                         den[:], start=True, stop=True)
                        rcpb = wp.tile([64, 512], F32, tag="rcpb", name="rcpb")
                        nc.vector.reciprocal_approx_fast(out=rcpb[:],
                                                         in_=psb[0:64, :])
                        dst = AOT[hp][r:r + 64, qc * 512:(qc + 1) * 512]
                        nc.vector.tensor_tensor(dst, pav[h][0:64, :],
                                                rcpb[:], ALU.mult)
                for s in range(qc * 4, qc * 4 + 4):
                    filler.append(("d", s, d_group(s)))
            emit_filler_until(lambda it: True)

    nc.compile()
    return nc


def _get_nc():
    if "nc" not in _CACHE:
        _CACHE["nc"] = _build()
    return _CACHE["nc"]


def _shard_inputs(x, Wq, bq, Wk, bk, Wv, bv, Wo):
    """Build the 8 per-core input maps (host-side shard/cast/transpose)."""
    x = np.asarray(x, np.float32)
    mask = np.triu(np.ones((128, 128), np.float32)).astype(BF16)  # [k, q] q>=k
    in_maps = []
    for c in range(8):
        b, hg = divmod(c, 2)
        dc = slice(hg * DC, (hg + 1) * DC)
        xT = np.ascontiguousarray(x[b].T).astype(BF16).reshape(EC, 128, S)
        wq_c = np.ascontiguousarray(Wq[:, dc]).astype(BF16).reshape(EC, 128, DC)
        wk_c = np.ascontiguousarray(Wk[:, dc]).astype(BF16).reshape(EC, 128, DC)
        wv_c = np.ascontiguousarray(Wv[:, dc]).astype(BF16).reshape(EC, 128, DC)
        wo_c = np.ascontiguousarray(Wo[dc, :]).astype(BF16).reshape(DC // 128, 128, E)
        bq_c = np.ascontiguousarray(np.asarray(bq[dc], np.float32).reshape(4, 128).T)
        bk_c = np.ascontiguousarray(np.asarray(bk[dc], np.float32).reshape(4, 128).T)
        bvb_c = np.ascontiguousarray(
            np.tile(np.asarray(bv[dc], np.float32).reshape(1, DC), (128, 1)))
        in_maps.append({
            "xT": xT, "wq": wq_c, "wk": wk_c, "wv": wv_c, "wo": wo_c,
            "bq": bq_c, "bk": bk_c, "bvb": bvb_c, "mask": mask,
        })
    return in_maps


def kernel(x, Wq, bq, Wk, bk, Wv, bv, Wo, bo):
    from concourse.bass_utils import run_bass_kernel_spmd

    nc = _get_nc()
    in_maps = _shard_inputs(x, Wq, bq, Wk, bk, Wv, bv, Wo)
    res = run_bass_kernel_spmd(nc, in_maps, core_ids=list(range(8)))
    bo = np.asarray(bo, np.float32)
    out = np.empty((B, S, E), np.float32)
    for b in range(B):
        p0 = res.results[2 * b]["out"].reshape(S, E)
        p1 = res.results[2 * b + 1]["out"].reshape(S, E)
        out[b] = p0 + p1 + bo
    return out
